# revision 92
# baseline (speedup 1.0000x reference)
import numpy as np
import ml_dtypes
import orjson

import concourse.bass as bass
import concourse.tile as tile
from concourse import mybir
import concourse.bass2jax as bass2jax
from concourse.bass_utils import run_bass_kernel_spmd

BF16 = ml_dtypes.bfloat16
E4M3 = ml_dtypes.float8_e4m3
E5M2 = ml_dtypes.float8_e5m2
JITTER = 0.01
T, H, F, E = 512, 1024, 4096, 8
CAP = 145          # padded tokens per expert per dispatch (max observed 145;
                   # overflow falls back to extra dispatches in kernel())
CAPA = 148         # fp8 storage stride: keeps engine-write slice offsets
                   # 4-byte aligned (CAP itself is odd)
NF = F // 128      # 32 f-tiles
NP = NF // 2       # 16 f-pairs (DoubleRow packs 2 contraction tiles)
NK = H // 128      # 8 k-tiles
NKP = NK // 2      # 4 k-pairs
WSEG = 3 * H       # per-f-tile packed weight row: [w1T | w3T | w2T]
PBLK = 2 * WSEG    # per-f-pair block
WS = 256.0         # weight scale: keeps fp8 values in the normal range

# fp8 error-compensated compute: every logical bf16 matmul pair becomes
# three fp8 DoubleRow matmuls (0.5 cyc/row each):
#   W@x ~= Q4(W*WS)@x8 + Q5(W*WS - Q4)@x8 + Q4(W*WS)@r8,  x = x8 + r8
# leaving only second-order (~1e-3) error at 0.75x the bf16 PE cost.


# ---------------------------------------------------------------------------
# walrus codegen rejects >1 sem wait on ANY instruction; Tile's scheduler
# freely assigns several. Post-process the final BIR (after all fuse passes)
# to hoist extra waits onto single-wait carrier instructions inserted just
# before the original on the same engine.
# ---------------------------------------------------------------------------
_MAX_WAITS = 1


def _split_multiwait(bir):
    n_clones = 0
    for fn in bir.get("functions", []):
        for blk in fn.get("blocks", []):
            out = []
            for inst in blk["instructions"]:
                si = inst.get("sync_info")
                if si and len(si.get("on_wait") or []) > _MAX_WAITS:
                    waits = si["on_wait"]
                    keep, rest = waits[-_MAX_WAITS:], waits[:-_MAX_WAITS]
                    for w in rest:
                        n_clones += 1
                        out.append({
                            "debug": inst.get("debug"),
                            "engine": inst["engine"],
                            "ins": [],
                            "is_reset_sema": False,
                            "name": f"{inst['name']}-w{n_clones}",
                            "opcode": "Drain",
                            "outs": [],
                            "sync_info": {"on_update": [],
                                          "on_wait": [w]},
                        })
                    si["on_wait"] = keep
                out.append(inst)
            blk["instructions"] = out
    return n_clones


def _install_bir_patch():
    if getattr(bass2jax.compile_bir_kernel, "_mw_patch", False):
        return
    _orig = bass2jax.compile_bir_kernel

    def _patched(bir_json, tmpdir, neff_name="file.neff"):
        bir = orjson.loads(bir_json)
        if _split_multiwait(bir):
            bir_json = orjson.dumps(bir)
        return _orig(bir_json, tmpdir, neff_name=neff_name)

    _patched._mw_patch = True
    bass2jax.compile_bir_kernel = _patched


# ---------------------------------------------------------------------------
# Host-side routing: exact fp32 replica of the reference phi_routing.
# ---------------------------------------------------------------------------
def _softmax_f32(logits):
    m = np.max(logits, axis=-1, keepdims=True)
    e = np.exp(logits - m, dtype=np.float32)
    return e / np.sum(e, axis=-1, keepdims=True)


def _routing(x, gate_w):
    logits = (x @ gate_w.T).astype(np.float32)          # [T,E]
    t_idx = np.arange(logits.shape[0])
    sel1 = np.argmax(logits, axis=1)
    m1 = logits[t_idx, sel1][:, None]
    factor1 = np.maximum(np.abs(logits), m1)
    mask1 = (m1 - logits) / factor1 > np.float32(2.0 * JITTER)
    p1 = _softmax_f32(np.where(mask1, -np.inf, logits).astype(np.float32))
    mult1 = p1[t_idx, sel1]

    l2 = logits.copy()
    l2[t_idx, sel1] = -np.inf
    sel2 = np.argmax(l2, axis=1)
    m2 = logits[t_idx, sel2][:, None]
    factor2 = np.maximum(np.abs(logits), m2)
    mask2 = (m2 - logits) / factor2 > np.float32(2.0 * JITTER)
    drop1 = np.zeros_like(mask2)
    drop1[t_idx, sel1] = True
    p2 = _softmax_f32(np.where(mask2 | drop1, -np.inf, logits).astype(np.float32))
    mult2 = p2[t_idx, sel2]
    return sel1, sel2, mult1.astype(np.float32), mult2.astype(np.float32)


# ---------------------------------------------------------------------------
# Device program: one expert per core, tokens padded to CAP.
#   xt8/xr8 [128, NK*CAP]  e4m3/e5m2   x8 / (x - x8), k-tile-major
#   wallW   [128, NP*PBLK] e4m3        Q4(W*WS) packed per f-pair
#   wallE   [128, NP*PBLK] e5m2        Q5(W*WS - Q4) same layout
#   y       [128, 8*CAP]   f32         256*(h @ w2.T).T, token dim free
# Per f-pair block layout (per partition row): [fi=0: w1 kt0..7 | w3 kt0..7 |
# w2 ht0..7, fi=1: same], 128 cols per chunk.
# ---------------------------------------------------------------------------
_PROGRAM = None

import os as _os
_HEADPIN = int(_os.environ.get("K_HEADPIN", "5"))
_W2DEF = int(_os.environ.get("K_W2DEF", "2"))
_G2DEPTH = int(_os.environ.get("K_G2DEPTH", "3"))
_SPRE = float(_os.environ.get("K_SPRE", "0.86"))


def _build_program():
    global _PROGRAM
    if _PROGRAM is not None:
        return _PROGRAM
    _install_bir_patch()
    nc = bass.Bass()
    e4 = mybir.dt.float8e4
    e5 = mybir.dt.float8e5
    bf = mybir.dt.bfloat16
    f32 = mybir.dt.float32
    DR = mybir.MatmulPerfMode.DoubleRow
    xt8 = nc.declare_dram_parameter("xt8", [128, NK * CAPA], e4,
                                    isOutput=False)
    xr8 = nc.declare_dram_parameter("xr8", [128, NK * CAPA], e5,
                                    isOutput=False)
    wallW = nc.declare_dram_parameter("wallW", [128, NP * PBLK], e4,
                                      isOutput=False)
    wallE = nc.declare_dram_parameter("wallE", [128, NP * PBLK], e5,
                                      isOutput=False)
    # exact bf16 w2 for the final f-pair: its G2 then needs no fp8 cast or
    # residual chain, shortening the kernel's tail critical path
    w2bf = nc.declare_dram_parameter("w2bf", [128, 2 * H], bf, isOutput=False)
    # y[p, b*CAP + c] = 256*out[b*128 + p, c]: one contiguous [128, 2*CAP]
    # DMA per psum bank; host un-permutes the h blocks
    y = nc.declare_dram_parameter("y", [128, 8 * CAP], f32, isOutput=True)

    # greedy DMA queue balancer: pick the least-loaded DMA-capable queue.
    # scalar (Act) also runs the silu/cast chain, pre-charged per pair below.
    qload = {"gpsimd": 0.0, "sync": 0.0, "scalar": 0.0}

    def q_issue(dur, prefer=None):
        name = prefer or min(qload, key=qload.get)
        qload[name] += dur
        return getattr(nc, name)

    with tile.TileContext(nc) as tc:
        with (tc.tile_pool(name="xp", bufs=1) as xp,
              tc.tile_pool(name="wp", bufs=8) as wp,
              tc.tile_pool(name="ep", bufs=8) as ep,
              tc.tile_pool(name="sp", bufs=3) as spool,
              tc.tile_pool(name="avp", bufs=5) as avp,
              tc.tile_pool(name="yo", bufs=1) as yo,
              tc.tile_pool(name="ps", bufs=2, space="PSUM") as ps,
              tc.tile_pool(name="py", bufs=1, space="PSUM") as py):
            xtile = xp.tile([128, NK, CAPA], e4)
            rtile = xp.tile([128, NK, CAPA], e5)
            # x8 on scalar (first), r8 early on sync behind pair0's first
            # E piece; G1/G3 run (W,x) then (E,x) then (W,r) sweeps so the
            # r-side may trail. Each transfer is under the 500ns descriptor
            # floor, so one DMA each.
            nc.scalar.dma_start(out=xtile[:], in_=xt8[:])
            qload["scalar"] += 0.5
            # silu table warm-up: allocated here, issued after pair 0's
            # critical head DMAs (the 1.6us table load must not block them)
            warm = spool.tile([128, 8], bf, name="warm")
            nc.vector.memset(warm[:], 0.0)
            # h-chunk -> (bank, slot): banks 0/1 hold 3 chunks, banks 2/3 one
            # each, so the last-closing banks stage and DMA out quickly
            psum_y = [py.tile([128, 512], f32, name=f"psum_y_{t}")
                      for t in range(4)]
            _CHUNK_BANK = {0: (0, 0), 1: (0, 1), 2: (0, 2),
                           3: (1, 0), 4: (1, 1), 5: (1, 2),
                           6: (2, 0), 7: (3, 0)}
            _BANK_FIRST_J = (0, 3, 6, 7)   # first chunk written per bank

            def g2dst(j):
                b, s = _CHUNK_BANK[j]
                return psum_y[b][:, s * CAP:(s + 1) * CAP]

            # G2 for one f-pair: 8 h-chunks x (W@h8, E@h8, W@hr8); each psum
            # bank's accumulation group starts on its first write (pair 0,
            # its first chunk) and stops on its last (set in the final flush)
            def emit_g2(av8_p, avr8_p, wcW_p, wcE_p, p_p, js):
                for j in js:
                    dst = g2dst(j)
                    first = p_p == 0 and j in _BANK_FIRST_J
                    nc.tensor.matmul(dst, lhsT=wcW_p[:, :, 16 + j, :],
                                     rhs=av8_p[:, :, :CAP], start=first,
                                     stop=False, perf_mode=DR)
                    nc.tensor.matmul(dst, lhsT=wcE_p[:, :, 16 + j, :],
                                     rhs=av8_p[:, :, :CAP], start=False,
                                     stop=False, perf_mode=DR)
                    nc.tensor.matmul(dst, lhsT=wcW_p[:, :, 16 + j, :],
                                     rhs=avr8_p[:, :, :CAP], start=False,
                                     stop=False, perf_mode=DR)

            def dma_w13(wc, blk, prefer=None):
                # w1+w3 segments of both f-tiles (needed when the pair's
                # G1/G3 run)
                for fi in range(2):
                    o = fi * WSEG
                    q_issue(0.79, prefer).dma_start(
                        out=wc[:, fi, 0:16, :], in_=blk[:, o:o + 2 * H])

            def dma_w2(wc, blk, prefer=None):
                # w2 segments (needed only by the pair's G2, ~2 pairs later)
                for fi in range(2):
                    o = fi * WSEG
                    q_issue(0.40, prefer).dma_start(
                        out=wc[:, fi, 16:, :], in_=blk[:, o + 2 * H:o + WSEG])

            pend = []  # deferred G2 work: (av8, avr8, wcW, wcE, p)
            w2_pend = []  # (wcW, blkW, wcE, blkE) w2-segment DMAs to issue
            av_f = {}  # final pair's bf16 av per f-tile (= WS * h)
            for p in range(NP):
                wcW = wp.tile([128, 2, 24, 128], e4)
                wcE = ep.tile([128, 2, 24, 128], e5)
                blkW = wallW[:, p * PBLK:(p + 1) * PBLK]
                blkE = wallE[:, p * PBLK:(p + 1) * PBLK]
                if p == 0:
                    # finest split: w1-f0 first so the very first matmul can
                    # start; r8 rides on sync between E pieces
                    nc.gpsimd.dma_start(out=wcW[:, 0, 0:8, :],
                                        in_=blkW[:, :H])
                    nc.sync.dma_start(out=wcE[:, 0, 0:8, :],
                                      in_=blkE[:, :H])
                    nc.sync.dma_start(out=wcW[:, 0, 8:16, :],
                                      in_=blkW[:, H:2 * H])
                    nc.gpsimd.dma_start(out=wcW[:, 1, 0:16, :],
                                        in_=blkW[:, WSEG:WSEG + 2 * H])
                    nc.scalar.dma_start(out=wcE[:, 0, 8:16, :],
                                        in_=blkE[:, H:2 * H])
                    nc.sync.dma_start(out=rtile[:], in_=xr8[:])
                    nc.scalar.dma_start(out=wcE[:, 1, 0:16, :],
                                        in_=blkE[:, WSEG:WSEG + 2 * H])
                    nc.scalar.activation(warm[:], warm[:],
                                         mybir.ActivationFunctionType.Silu)
                    qload["gpsimd"] += 1.29
                    qload["sync"] += 1.5
                    qload["scalar"] += 3.55
                elif p <= _HEADPIN:
                    # head is zero-slack: dedicate gpsimd to the W stream and
                    # sync to the E stream, in exact consumption order
                    dma_w13(wcW, blkW, prefer="gpsimd")
                    dma_w13(wcE, blkE, prefer="sync")
                else:
                    dma_w13(wcW, blkW)
                    dma_w13(wcE, blkE)
                w2_pend.append((wcW, blkW, wcE, blkE))
                if p >= _W2DEF:
                    wcW_o, blkW_o, wcE_o, blkE_o = w2_pend.pop(0)
                    dma_w2(wcW_o, blkW_o)
                    dma_w2(wcE_o, blkE_o)
                if p == NP - 3:
                    # exact bf16 w2 for the final pair (used by its G2)
                    wbf = yo.tile([128, 2, 8, 128], bf, name="wbf")
                    q_issue(1.58).dma_start(out=wbf[:], in_=w2bf[:])
                lastp = p == NP - 1
                if not lastp:
                    av8 = avp.tile([128, 2, CAPA], e4)
                    avr8 = avp.tile([128, 2, CAPA], e5)
                # one 2-bank psum tile per pair: bank fi holds p1|p3
                pt = ps.tile([128, 2, 512], f32)
                for fi in range(2):
                    p1 = pt[:, fi, 0:CAP]
                    p3 = pt[:, fi, CAPA:CAPA + CAP]

                    # p1/p3 share a bank: single group spanning G1+G3 (start
                    # on G1's first matmul zeroes both halves, stop on the
                    # group's final matmul). Sweeps ordered (W,x8) (E,x8)
                    # (W,r8) so the E/r streams may trail the W/x ones.
                    def sweep(seg, base, wall, xin, start=False, stop=False):
                        for kp in range(NKP):
                            nc.tensor.matmul(
                                seg,
                                lhsT=wall[:, fi, base + 2 * kp:base + 2 * kp + 2, :],
                                rhs=xin[:, 2 * kp:2 * kp + 2, :CAP],
                                start=(start and kp == 0),
                                stop=(stop and kp == NKP - 1), perf_mode=DR)

                    if p == 0 and fi == 0:
                        # both r8 sweeps last: r8 is the latest head arrival
                        sweep(p1, 0, wcW, xtile, start=True)
                        sweep(p1, 0, wcE, xtile)
                        sweep(p3, 8, wcW, xtile)
                        sweep(p3, 8, wcE, xtile)
                        sweep(p1, 0, wcW, rtile)
                        sweep(p3, 8, wcW, rtile, stop=True)
                    else:
                        sweep(p1, 0, wcW, xtile, start=True)
                        sweep(p1, 0, wcE, xtile)
                        sweep(p1, 0, wcW, rtile)
                        # deferred G2 of an earlier pair rides between G1/G3
                        if len(pend) > _G2DEPTH:
                            emit_g2(*pend.pop(0), range(8))
                        sweep(p3, 8, wcW, xtile)
                        sweep(p3, 8, wcE, xtile)
                        sweep(p3, 8, wcW, rtile, stop=True)
                    if lastp:
                        # final pair: exact bf16 w2 G2 follows, so only
                        # silu+mul here (no fp8 cast/residual chain); av_f
                        # carries the WS scale, w2bf is plain w2
                        s1 = spool.tile([128, CAPA], bf)
                        nc.scalar.activation(
                            s1[:, :CAP], p1,
                            mybir.ActivationFunctionType.Silu, scale=1.0 / WS)
                        av_f[fi] = spool.tile([128, CAPA], bf,
                                              name=f"av_f{fi}")
                        nc.vector.tensor_mul(av_f[fi][:, :CAP],
                                             s1[:, :CAP], p3)
                if not lastp:
                    # pair-granular activation chain over both f-tiles at
                    # once; psum holds WS-scaled values: p1 = WS*a, p3 = WS*b
                    s1 = spool.tile([128, 2, CAPA], bf)
                    nc.scalar.activation(s1[:, :, :CAP], pt[:, :, 0:CAP],
                                         mybir.ActivationFunctionType.Silu,
                                         scale=1.0 / WS)
                    av = spool.tile([128, 2, CAPA], bf)      # = WS * h
                    nc.vector.tensor_mul(av[:, :, :CAP], s1[:, :, :CAP],
                                         pt[:, :, CAPA:CAPA + CAP])
                    nc.scalar.mul(av8[:, :, :CAP], av[:, :, :CAP], 1.0 / WS)
                    nc.vector.scalar_tensor_tensor(
                        avr8[:, :, :CAP], av[:, :, :CAP], 1.0 / WS,
                        av8[:, :, :CAP],
                        mybir.AluOpType.mult, mybir.AluOpType.subtract)
                qload["scalar"] += _SPRE  # silu + cast ride the Act queue
                if not lastp:
                    pend.append((av8, avr8, wcW, wcE, p))
            # w2 segments of the second-to-last pair; the final pair's G2
            # uses the exact bf16 w2bf, so its fp8 w2 segments are never read
            wcW_o, blkW_o, wcE_o, blkE_o = w2_pend.pop(0)
            dma_w2(wcW_o, blkW_o)
            dma_w2(wcE_o, blkE_o)
            # flush: the older pairs entirely (independent of the final
            # pair's activation chain, hiding its latency), then the final
            # pair per bank in exact bf16 with the bank's stop on its last
            # matmul
            for args in pend:
                emit_g2(*args, range(8))
            _BANK_CHUNKS = ((0, 1, 2), (3, 4, 5), (6,), (7,))
            for b in range(4):
                for j in _BANK_CHUNKS[b]:
                    for fi in range(2):
                        nc.tensor.matmul(
                            g2dst(j), lhsT=wbf[:, fi, j, :],
                            rhs=av_f[fi][:, :CAP], start=False,
                            stop=(j == _BANK_CHUNKS[b][-1] and fi == 1))
            # stage each bank to SBUF (DMA cannot read PSUM), then DMA out.
            # Banks close in order 0..3 with sizes 3/2/2/1 chunks; banks 2+3
            # share one staging tile and one DMA so only three y DMAs are
            # issued (one per DMA-capable queue, no second-slot serialization)
            yt0 = yo.tile([128, 3 * CAP], f32, name="yt0")
            nc.vector.tensor_copy(yt0[:], psum_y[0][:, 0:3 * CAP])
            nc.gpsimd.dma_start(out=y[:, 0:3 * CAP], in_=yt0[:])
            yt1 = yo.tile([128, 3 * CAP], f32, name="yt1")
            nc.scalar.copy(yt1[:], psum_y[1][:, 0:3 * CAP])
            nc.sync.dma_start(out=y[:, 3 * CAP:6 * CAP], in_=yt1[:])
            yt23 = yo.tile([128, 2 * CAP], f32, name="yt23")
            nc.vector.tensor_copy(yt23[:, 0:CAP], psum_y[2][:, 0:CAP])
            nc.scalar.copy(yt23[:, CAP:2 * CAP], psum_y[3][:, 0:CAP])
            nc.scalar.dma_start(out=y[:, 6 * CAP:8 * CAP], in_=yt23[:])
    _PROGRAM = nc
    return nc


# ---------------------------------------------------------------------------
# Host-side data marshalling
# ---------------------------------------------------------------------------
def _pack_weights(w1e, w2e, w3e):
    # per f-tile packed rows [128, 3H]: w1 lhsT k-chunks | w3 | w2T h-chunks
    w1b = np.ascontiguousarray(
        w1e.reshape(NF, 128, NK, 128).transpose(0, 3, 2, 1)).reshape(NF, 128, H)
    w3b = np.ascontiguousarray(
        w3e.reshape(NF, 128, NK, 128).transpose(0, 3, 2, 1)).reshape(NF, 128, H)
    w2b = np.ascontiguousarray(w2e.T).reshape(NF, 128, H)
    seg = np.concatenate([w1b, w3b, w2b], axis=2)      # [NF, 128, 3H]
    # f-pair blocks: [NP, 128, 2, 3H] -> [128, NP*PBLK]
    pair = seg.reshape(NP, 2, 128, WSEG).transpose(2, 0, 1, 3)
    full = np.ascontiguousarray(pair).reshape(128, NP * PBLK)
    fs = full * np.float32(WS)
    W8 = fs.astype(E4M3)
    E8 = (fs - W8.astype(np.float32)).astype(E5M2)
    # exact (unscaled) bf16 w2 for the final f-pair: [128, 2(fi), 8(j), 128]
    w2last = np.ascontiguousarray(
        w2b[NF - 2:].transpose(1, 0, 2)).reshape(128, 2 * H).astype(BF16)
    return W8, E8, w2last


def _pack_tokens(x32, idx):
    xg = np.zeros((CAPA, H), dtype=np.float32)
    xg[:len(idx)] = x32[idx]
    xk = np.ascontiguousarray(
        xg.reshape(CAPA, NK, 128).transpose(2, 1, 0)).reshape(128, NK * CAPA)
    x8 = xk.astype(E4M3)
    r8 = (xk - x8.astype(np.float32)).astype(E5M2)
    return x8, r8


def kernel(hidden_states, gate_w, w1, w2, w3):
    B, S, _ = hidden_states.shape
    x = np.asarray(hidden_states, dtype=np.float32).reshape(-1, H)
    sel1, sel2, mult1, mult2 = _routing(x, np.asarray(gate_w, np.float32))

    idx_e, wgt_e = [], []
    for e in range(E):
        idx = np.where((sel1 == e) | (sel2 == e))[0]
        idx_e.append(idx)
        wgt_e.append(np.where(sel1[idx] == e, mult1[idx], mult2[idx]))

    nc = _build_program()
    walls = [_pack_weights(np.asarray(w1[e], np.float32),
                           np.asarray(w2[e], np.float32),
                           np.asarray(w3[e], np.float32)) for e in range(E)]

    n_runs = max(1, max((len(i) + CAP - 1) // CAP for i in idx_e))
    out = np.zeros((T, H), dtype=np.float32)
    for r in range(n_runs):
        in_maps = []
        chunks = []
        for e in range(E):
            chunk = idx_e[e][r * CAP:(r + 1) * CAP]
            chunks.append(chunk)
            x8, r8 = _pack_tokens(x, chunk)
            in_maps.append({"xt8": x8, "xr8": r8,
                            "wallW": walls[e][0], "wallE": walls[e][1],
                            "w2bf": walls[e][2]})
        res = run_bass_kernel_spmd(nc, in_maps, core_ids=list(range(E)))
        for e in range(E):
            chunk = chunks[e]
            if len(chunk) == 0:
                continue
            w = wgt_e[e][r * CAP:(r + 1) * CAP] / np.float32(WS)
            yf = res.results[e]["y"].reshape(128, 8, CAP) \
                .swapaxes(0, 1).reshape(H, CAP)
            out[chunk] += w[:, None] * yf[:, :len(chunk)].T
    return out.reshape(B, S, H)
